# revision 2
# baseline (speedup 1.0000x reference)
"""3-layer GCN on 8 Trainium2 NeuronCores (SPMD, Bass/Tile) — v3.

Like v2 (degree-sorted blocks, slot-aligned edges, bf16 tables, DVE
tree segment-sum) but the per-edge row fetch uses gpsimd.dma_gather
(InstDMAGatherAnt) over NODE-PAIR granules: the AllGathered support
table is viewed as [NTOT/2, 128] bf16 (256B rows, pair-id < 25088 fits
the instruction's int16 indices). Parity selection of the wanted half
is folded into the duplicated edge-weight multiply (unwanted half gets
weight 0) and the first tree-combine level.
"""
import numpy as np
import ml_dtypes

N_NODES = 50000
N_EDGES = 800000
NFEAT, NHID, NCLASS = 512, 64, 40
NCORES = 8
P = 128
BLOCKS = 49                 # rounds (blocks per core)
NLOC = BLOCKS * P           # 6272 rows per core
NB = NCORES * BLOCKS        # 392 blocks total
NTOT = NCORES * NLOC        # 50176 padded nodes
NPAIR = NTOT // 2
GROUP_ROUNDS = [1, 2, 3, 5, 6, 8, 8, 8, 8]   # rounds per gather group

_cache = {}


def _partition_graph(row, col, edge_weight):
    """Degree-sorted blocks + snake deal; slot-aligned edge packing.

    Returns perm (old node id -> new global id), per-core idx16/ewp4
    arrays, and the uniform group profile [(R_g, u_g), ...].
    """
    deg = np.bincount(row, minlength=N_NODES)
    deg_full = np.concatenate([deg, np.zeros(NTOT - N_NODES, np.int64)])
    order = np.argsort(-deg_full, kind="stable")      # rank -> node
    inv = np.empty(NTOT, np.int64)
    inv[order] = np.arange(NTOT)                      # node -> rank

    rank = inv[:N_NODES]
    blk = rank // P
    pos = rank % P
    rnd = blk // NCORES
    j = blk % NCORES
    core = np.where(rnd % 2 == 0, j, NCORES - 1 - j)  # snake deal
    perm_real = core * NLOC + rnd * P + pos

    u_r = np.maximum(1, deg_full[order[(np.arange(BLOCKS) * NCORES) * P]])
    profile = []
    base_r = np.zeros(BLOCKS, np.int64)
    cofs = 0
    rs = 0
    for R in GROUP_ROUNDS:
        ug = max(int(u_r[rs:rs + R].max()), 2)
        profile.append((R, ug))
        for r in range(rs, rs + R):
            base_r[r] = cofs + (r - rs) * ug
        cofs += R * ug
        rs += R
    SC = cofs

    g_r = perm_real[row]
    g_c = perm_real[col]
    e_core = g_r // NLOC
    e_rnd = (g_r % NLOC) // P
    e_pos = g_r % P

    order_e = np.argsort(g_r, kind="stable")
    g_r_s = g_r[order_e]
    starts = np.searchsorted(g_r_s, g_r_s, side="left")
    k = np.arange(N_EDGES, dtype=np.int64) - starts
    colidx = base_r[e_rnd[order_e]] + k
    ec = e_core[order_e]
    ep = e_pos[order_e]
    pair = (g_c[order_e] // 2).astype(np.int16)
    par = (g_c[order_e] % 2).astype(np.int64)
    w = np.asarray(edge_weight, np.float32)[order_e]

    # idx16: [NCORES, 16, 8*SC]; slot i=colidx*128+p -> [i%16, i//16]
    idx16 = np.zeros((NCORES, 16, 8 * SC), np.int16)
    idx16[ec, ep % 16, 8 * colidx + ep // 16] = pair
    idx16 = np.tile(idx16, (1, 8, 1))                 # replicate to 128 rows

    # ewp4: [NCORES, 128, 4*SC]; slot (p, c): [w*(par==0)]*2 + [w*(par==1)]*2
    ewp4 = np.zeros((NCORES, P, 4 * SC), np.float32)
    ewp4[ec, ep, 4 * colidx + 2 * par] = w
    ewp4[ec, ep, 4 * colidx + 2 * par + 1] = w
    ewp4 = ewp4.astype(ml_dtypes.bfloat16)

    return perm_real, idx16, ewp4, tuple(profile), SC


def _build_program(profile, SC):
    import concourse.bacc as bacc
    import concourse.bass as bass
    import concourse.mybir as mybir
    import concourse.tile as tile

    f32 = mybir.dt.float32
    bf16 = mybir.dt.bfloat16
    i16 = mybir.dt.int16
    AX = mybir.AxisListType.X
    AF = mybir.ActivationFunctionType
    OP = mybir.AluOpType

    nc = bacc.Bacc("TRN2", target_bir_lowering=False, debug=False,
                   num_devices=NCORES,
                   dynamic_dma_scratch_size=49152, num_swdge_queues=4)
    xT = nc.dram_tensor("xT", [NFEAT, NLOC], bf16, kind="ExternalInput").ap()
    idxt = nc.dram_tensor("idxt", [P, 8 * SC], i16, kind="ExternalInput").ap()
    ewp = nc.dram_tensor("ewp", [P, 4 * SC], bf16, kind="ExternalInput").ap()
    W1 = nc.dram_tensor("W1", [NFEAT, NHID], bf16, kind="ExternalInput").ap()
    W2 = nc.dram_tensor("W2", [NHID, NHID], bf16, kind="ExternalInput").ap()
    W3p = nc.dram_tensor("W3p", [NHID, NHID], bf16, kind="ExternalInput").ap()
    b1r = nc.dram_tensor("b1r", [P, NHID], f32, kind="ExternalInput").ap()
    b2r = nc.dram_tensor("b2r", [P, NHID], f32, kind="ExternalInput").ap()
    b3r = nc.dram_tensor("b3r", [P, NHID], f32, kind="ExternalInput").ap()
    ident = nc.dram_tensor("ident", [P, P], f32, kind="ExternalInput").ap()
    out = nc.dram_tensor("out", [NLOC, NCLASS], f32, kind="ExternalOutput").ap()

    rg = [list(range(NCORES))]

    with tile.TileContext(nc) as tc:
        with (
            tc.tile_pool(name="consts", bufs=1) as cp,
            tc.tile_pool(name="dram", bufs=1, space="DRAM") as dp,
            tc.tile_pool(name="msg", bufs=2) as mp,
            tc.tile_pool(name="hall", bufs=2) as hp,
            tc.tile_pool(name="supp", bufs=4) as supp,
            tc.tile_pool(name="soft", bufs=2) as sfp,
            tc.tile_pool(name="ps_dense", bufs=2, space="PSUM") as ps_dense,
            tc.tile_pool(name="ps_tr", bufs=2, space="PSUM") as ps_tr,
        ):
            # ---- constants ----
            idx_sb = cp.tile([P, 8 * SC], i16)
            ewp_sb = cp.tile([P, 4 * SC], bf16)
            ident_sb = cp.tile([P, P], f32)
            b1_sb = cp.tile([P, NHID], f32)
            b2_sb = cp.tile([P, NHID], f32)
            b3_sb = cp.tile([P, NHID], f32)
            W2_sb = cp.tile([NHID, NHID], bf16)
            W3_sb = cp.tile([NHID, NHID], bf16)
            nc.sync.dma_start(out=idx_sb[:], in_=idxt[:])
            nc.sync.dma_start(out=ewp_sb[:], in_=ewp[:])
            nc.sync.dma_start(out=ident_sb[:], in_=ident[:])
            nc.sync.dma_start(out=b1_sb[:], in_=b1r[:])
            nc.sync.dma_start(out=b2_sb[:], in_=b2r[:])
            nc.sync.dma_start(out=b3_sb[:], in_=b3r[:])
            nc.sync.dma_start(out=W2_sb[:], in_=W2[:])
            nc.sync.dma_start(out=W3_sb[:], in_=W3p[:])
            W1_sb = []
            for kc in range(4):
                w = cp.tile([P, NHID], bf16, name=f"W1_sb_{kc}")
                nc.sync.dma_start(out=w[:], in_=W1[kc * P:(kc + 1) * P, :])
                W1_sb.append(w)

            hT1_sb = cp.tile([NHID, NLOC], bf16)
            hT2_sb = cp.tile([NHID, NLOC], bf16)

            # ---- internal DRAM (tables viewed as pair rows) ----
            sup1_l = dp.tile([NLOC, NHID], bf16)
            sup2_l = dp.tile([NLOC, NHID], bf16)
            sup3_l = dp.tile([NLOC, NHID], bf16)
            T1 = dp.tile([NPAIR, 2 * NHID], bf16, addr_space="Shared")
            T2 = dp.tile([NPAIR, 2 * NHID], bf16, addr_space="Shared")
            T3 = dp.tile([NPAIR, 2 * NHID], bf16, addr_space="Shared")

            # ---- phase A: support1 = x @ W1 + b1 ----
            with tc.tile_pool(name="xchunk", bufs=8) as xp:
                for m in range(BLOCKS):
                    chunks = []
                    for kc in range(4):
                        s = xp.tile([P, P], bf16, name="xc")
                        nc.sync.dma_start(
                            out=s[:],
                            in_=xT[kc * P:(kc + 1) * P, m * P:(m + 1) * P])
                        chunks.append(s)
                    psd = ps_dense.tile([P, NHID], f32, space="PSUM")
                    for kc in range(4):
                        nc.tensor.matmul(
                            psd[:], lhsT=chunks[kc][:],
                            rhs=W1_sb[kc][:], start=(kc == 0), stop=(kc == 3))
                    sup = supp.tile([P, NHID], bf16)
                    nc.vector.tensor_tensor(out=sup[:], in0=psd[:], in1=b1_sb[:],
                                            op=OP.add)
                    nc.sync.dma_start(out=sup1_l[m * P:(m + 1) * P, :], in_=sup[:])

            nc.gpsimd.collective_compute(
                "AllGather", OP.bypass, replica_groups=rg,
                ins=[sup1_l.opt()], outs=[T1.opt()])

            JMAX = max(R * u for (R, u) in profile)
            RMAX = max(R for (R, u) in profile)

            def spmm(table, dout, post_group):
                """Pair-gather + parity-masked scale + tree segment-sum."""
                cofs = 0
                rstart = 0
                if not hasattr(spmm, "qn"):
                    spmm.qn = 0
                for (R, u) in profile:
                    J = R * u
                    msg = mp.tile([P, JMAX * 2 * NHID], bf16, name="msg_t")
                    # SWDGE ring fits ~24*J+19 descs of 3072; split gathers
                    nsub = -(-J // 40)
                    step = -(-J // nsub)
                    for a in range(0, J, step):
                        b = min(a + step, J)
                        nc.gpsimd.dma_gather(
                            out_ap=msg[:, (a * 2 * NHID):(b * 2 * NHID)]
                            .rearrange("p (j e) -> p j e", e=2 * NHID),
                            in_ap=table[:, :],
                            idxs_ap=idx_sb[:, 8 * (cofs + a):8 * (cofs + b)],
                            num_idxs=P * (b - a), num_idxs_reg=P * (b - a),
                            elem_size=2 * NHID, single_packet=False,
                            queue_num=spmm.qn % 4)
                        spmm.qn += 1
                    # scale both halves by parity-masked duplicated weights
                    # (one 3-free-dim instruction per pair-half)
                    mj0 = msg[:, :J * 2 * NHID].rearrange(
                        "p (j e) -> p j e", e=2 * NHID)
                    for hh in range(2):
                        m4h = mj0[:, :, hh * NHID:hh * NHID + dout].rearrange(
                            "p j (f2 r) -> p j f2 r", r=2)
                        e4h = ewp_sb[:, 4 * cofs:4 * (cofs + J)].rearrange(
                            "p (j h r) -> p j h r", h=2, r=2)[
                            :, :, hh, :].unsqueeze(2).to_broadcast(
                            [P, J, dout // 2, 2])
                        nc.vector.tensor_tensor(out=m4h, in0=m4h, in1=e4h,
                                                op=OP.mult)
                    # combine halves: wanted half has the weight, other is 0
                    mj = msg[:, :J * 2 * NHID].rearrange(
                        "p (j e) -> p j e", e=2 * NHID)
                    nc.vector.tensor_tensor(
                        out=mj[:, :, 0:dout], in0=mj[:, :, 0:dout],
                        in1=mj[:, :, NHID:NHID + dout], op=OP.add)
                    # in-place pairwise tree over the u occurrence columns
                    m3 = msg[:, :J * 2 * NHID].rearrange(
                        "p (r j e) -> p r j e", r=R, j=u)
                    h = u
                    while h > 2:
                        k = h - h // 2
                        nc.vector.tensor_tensor(
                            out=m3[:, :, 0:h // 2, 0:dout],
                            in0=m3[:, :, 0:h // 2, 0:dout],
                            in1=m3[:, :, k:k + h // 2, 0:dout], op=OP.add)
                        h = k
                    h_all = hp.tile([P, RMAX * NHID], f32, name="h_all")
                    ha = h_all[:, :R * NHID].rearrange(
                        "p (r f) -> p r f", f=NHID)[:, :, 0:dout]
                    if h == 2:
                        nc.vector.tensor_tensor(
                            out=ha, in0=m3[:, :, 0, 0:dout],
                            in1=m3[:, :, 1, 0:dout], op=OP.add)
                    else:
                        nc.vector.tensor_scalar_mul(ha, m3[:, :, 0, 0:dout], 1.0)
                    post_group(rstart, R, h_all)
                    cofs += J
                    rstart += R

            def mk_post_h(hT_dst, func, Wn_sb, bn_sb, supn_l, dn):
                def post(rstart, R, h_all):
                    for i in range(R):
                        r = rstart + i
                        pst = ps_tr.tile([NHID, P], f32, space="PSUM",
                                         name="ps_t")
                        nc.tensor.transpose(
                            pst[:], h_all[:, i * NHID:(i + 1) * NHID],
                            ident_sb[:])
                        nc.scalar.activation(
                            hT_dst[:, r * P:(r + 1) * P], pst[:], func)
                        psd = ps_dense.tile([P, NHID], f32, space="PSUM")
                        nc.tensor.matmul(
                            psd[:], lhsT=hT_dst[:, r * P:(r + 1) * P],
                            rhs=Wn_sb[:], start=True, stop=True)
                        sup = supp.tile([P, NHID], bf16, name="sup_n")
                        nc.vector.tensor_tensor(
                            out=sup[:], in0=psd[:], in1=bn_sb[:], op=OP.add)
                        nc.sync.dma_start(
                            out=supn_l[r * P:(r + 1) * P, :], in_=sup[:])
                return post

            # ---- L1 SPMM + relu -> hT1, dense2 -> sup2 ----
            spmm(T1, NHID, mk_post_h(hT1_sb, AF.Relu, W2_sb, b2_sb,
                                     sup2_l, NHID))
            nc.gpsimd.collective_compute(
                "AllGather", OP.bypass, replica_groups=rg,
                ins=[sup2_l.opt()], outs=[T2.opt()])

            # ---- L2 SPMM -> hT2, dense3 (padded) -> sup3 ----
            spmm(T2, NHID, mk_post_h(hT2_sb, AF.Copy, W3_sb, b3_sb,
                                     sup3_l, NHID))
            nc.gpsimd.collective_compute(
                "AllGather", OP.bypass, replica_groups=rg,
                ins=[sup3_l.opt()], outs=[T3.opt()])

            # ---- L3 SPMM + log_softmax -> out ----
            def post3(rstart, R, h_all):
                h3 = h_all[:, :R * NHID].rearrange(
                    "p (r f) -> p r f", f=NHID)[:, :, 0:NCLASS]
                negmax = sfp.tile([P, RMAX], f32, name="negmax")
                nc.vector.reduce_max(out=negmax[:, :R], in_=h3, axis=AX,
                                     negate=True)
                t = sfp.tile([P, RMAX * NCLASS], f32, name="t_sm")
                tv = t[:, :R * NCLASS].rearrange("p (r f) -> p r f", f=NCLASS)
                nc.vector.tensor_tensor(
                    out=tv, in0=h3,
                    in1=negmax[:, :R].unsqueeze(2).to_broadcast([P, R, NCLASS]),
                    op=OP.add)
                e = sfp.tile([P, RMAX * NCLASS], f32, name="e_sm")
                nc.scalar.activation(e[:, :R * NCLASS], t[:, :R * NCLASS],
                                     AF.Exp)
                ssum = sfp.tile([P, RMAX], f32, name="ssum")
                nc.vector.reduce_sum(
                    out=ssum[:, :R],
                    in_=e[:, :R * NCLASS].rearrange("p (r f) -> p r f",
                                                    f=NCLASS),
                    axis=AX)
                lns = sfp.tile([P, RMAX], f32, name="lns")
                nc.scalar.activation(lns[:, :R], ssum[:, :R], AF.Ln)
                o = sfp.tile([P, RMAX * NCLASS], f32, name="o_sm")
                ov = o[:, :R * NCLASS].rearrange("p (r f) -> p r f", f=NCLASS)
                nc.vector.tensor_tensor(
                    out=ov, in0=tv,
                    in1=lns[:, :R].unsqueeze(2).to_broadcast([P, R, NCLASS]),
                    op=OP.subtract)
                nc.sync.dma_start(
                    out=out[rstart * P:(rstart + R) * P, :].rearrange(
                        "(r p) f -> p r f", p=P),
                    in_=ov)
            spmm(T3, NCLASS, post3)

    nc.compile()
    return nc


def kernel(x, edge_weight, W1, b1, W2, b2, W3, b3, row, col):
    from concourse import bass_utils

    x = np.asarray(x, np.float32)
    edge_weight = np.asarray(edge_weight, np.float32)
    row = np.asarray(row, np.int64)
    col = np.asarray(col, np.int64)

    perm, idx16, ewp4, profile, SC = _partition_graph(
        row, col, edge_weight)

    key = (profile, SC)
    if key not in _cache:
        _cache[key] = _build_program(profile, SC)
    nc = _cache[key]

    x_new = np.zeros((NTOT, NFEAT), np.float32)
    x_new[perm] = x
    ident = np.eye(P, dtype=np.float32)
    bf = ml_dtypes.bfloat16
    W3p = np.zeros((NHID, NHID), np.float32)
    W3p[:, :NCLASS] = np.asarray(W3, np.float32)
    b3p = np.zeros((NHID,), np.float32)
    b3p[:NCLASS] = np.asarray(b3, np.float32)
    in_maps = []
    for c in range(NCORES):
        in_maps.append({
            "xT": np.ascontiguousarray(
                x_new[c * NLOC:(c + 1) * NLOC].T).astype(bf),
            "idxt": np.ascontiguousarray(idx16[c]),
            "ewp": np.ascontiguousarray(ewp4[c]),
            "W1": np.asarray(W1, np.float32).astype(bf),
            "W2": np.asarray(W2, np.float32).astype(bf),
            "W3p": W3p.astype(bf),
            "b1r": np.tile(np.asarray(b1, np.float32), (P, 1)),
            "b2r": np.tile(np.asarray(b2, np.float32), (P, 1)),
            "b3r": np.tile(b3p, (P, 1)),
            "ident": ident,
        })

    res = bass_utils.run_bass_kernel_spmd(
        nc, in_maps, core_ids=list(range(NCORES)),
        trace=kernel.trace)
    kernel.last_result = res

    full = np.concatenate([res.results[c]["out"] for c in range(NCORES)],
                          axis=0)
    return full[perm].astype(np.float32)


kernel.trace = False
kernel.last_result = None


# revision 3
# speedup vs baseline: 1.2114x; 1.2114x over previous
"""3-layer GCN on 8 Trainium2 NeuronCores (SPMD, Bass/Tile) — v3.

Like v2 (degree-sorted blocks, slot-aligned edges, bf16 tables, DVE
tree segment-sum) but the per-edge row fetch uses gpsimd.dma_gather
(InstDMAGatherAnt) over NODE-PAIR granules: the AllGathered support
table is viewed as [NTOT/2, 128] bf16 (256B rows, pair-id < 25088 fits
the instruction's int16 indices). Parity selection of the wanted half
is folded into the duplicated edge-weight multiply (unwanted half gets
weight 0) and the first tree-combine level.
"""
import numpy as np
import ml_dtypes

N_NODES = 50000
N_EDGES = 800000
NFEAT, NHID, NCLASS = 512, 64, 40
NCORES = 8
P = 128
BLOCKS = 49                 # rounds (blocks per core)
NLOC = BLOCKS * P           # 6272 rows per core
NB = NCORES * BLOCKS        # 392 blocks total
NTOT = NCORES * NLOC        # 50176 padded nodes
NPAIR = NTOT // 2
JCAP = 40                   # max R*u per gather group (ring limit)

_cache = {}


def _partition_graph(row, col, edge_weight):
    """Degree-sorted blocks + snake deal; slot-aligned edge packing.

    Returns perm (old node id -> new global id), per-core idx16/ewp4
    arrays, and the uniform group profile [(R_g, u_g), ...].
    """
    deg = np.bincount(row, minlength=N_NODES)
    deg_full = np.concatenate([deg, np.zeros(NTOT - N_NODES, np.int64)])
    order = np.argsort(-deg_full, kind="stable")      # rank -> node
    inv = np.empty(NTOT, np.int64)
    inv[order] = np.arange(NTOT)                      # node -> rank

    rank = inv[:N_NODES]
    blk = rank // P
    pos = rank % P
    rnd = blk // NCORES
    j = blk % NCORES
    core = np.where(rnd % 2 == 0, j, NCORES - 1 - j)  # snake deal
    perm_real = core * NLOC + rnd * P + pos

    u_r = np.maximum(1, deg_full[order[(np.arange(BLOCKS) * NCORES) * P]])
    groups = []                       # greedy: grow R while R*max(u) <= JCAP
    rs = 0
    while rs < BLOCKS:
        R = 1
        while rs + R < BLOCKS and (R + 1) * max(
                int(u_r[rs:rs + R + 1].max()), 2) <= JCAP:
            R += 1
        groups.append(R)
        rs += R
    profile = []
    base_r = np.zeros(BLOCKS, np.int64)
    cofs = 0
    rs = 0
    for R in groups:
        ug = max(int(u_r[rs:rs + R].max()), 2)
        profile.append((R, ug))
        for r in range(rs, rs + R):
            base_r[r] = cofs + (r - rs) * ug
        cofs += R * ug
        rs += R
    SC = cofs

    g_r = perm_real[row]
    g_c = perm_real[col]
    e_core = g_r // NLOC
    e_rnd = (g_r % NLOC) // P
    e_pos = g_r % P

    order_e = np.argsort(g_r, kind="stable")
    g_r_s = g_r[order_e]
    starts = np.searchsorted(g_r_s, g_r_s, side="left")
    k = np.arange(N_EDGES, dtype=np.int64) - starts
    colidx = base_r[e_rnd[order_e]] + k
    ec = e_core[order_e]
    ep = e_pos[order_e]
    pair = (g_c[order_e] // 2).astype(np.int16)
    par = (g_c[order_e] % 2).astype(np.int64)
    w = np.asarray(edge_weight, np.float32)[order_e]

    # idx16: [NCORES, 16, 8*SC]; slot i=colidx*128+p -> [i%16, i//16]
    idx16 = np.zeros((NCORES, 16, 8 * SC), np.int16)
    idx16[ec, ep % 16, 8 * colidx + ep // 16] = pair
    idx16 = np.tile(idx16, (1, 8, 1))                 # replicate to 128 rows

    # ewp4: [NCORES, 128, 4*SC]; slot (p, c): [w*(par==0)]*2 + [w*(par==1)]*2
    ewp4 = np.zeros((NCORES, P, 4 * SC), np.float32)
    ewp4[ec, ep, 4 * colidx + 2 * par] = w
    ewp4[ec, ep, 4 * colidx + 2 * par + 1] = w
    ewp4 = ewp4.astype(ml_dtypes.bfloat16)

    return perm_real, idx16, ewp4, tuple(profile), SC


def _build_program(profile, SC):
    import concourse.bacc as bacc
    import concourse.bass as bass
    import concourse.mybir as mybir
    import concourse.tile as tile

    f32 = mybir.dt.float32
    bf16 = mybir.dt.bfloat16
    i16 = mybir.dt.int16
    AX = mybir.AxisListType.X
    AF = mybir.ActivationFunctionType
    OP = mybir.AluOpType

    nc = bacc.Bacc("TRN2", target_bir_lowering=False, debug=False,
                   num_devices=NCORES,
                   dynamic_dma_scratch_size=49152, num_swdge_queues=4)
    xT = nc.dram_tensor("xT", [NFEAT, NLOC], bf16, kind="ExternalInput").ap()
    idxt = nc.dram_tensor("idxt", [P, 8 * SC], i16, kind="ExternalInput").ap()
    ewp = nc.dram_tensor("ewp", [P, 4 * SC], bf16, kind="ExternalInput").ap()
    W1 = nc.dram_tensor("W1", [NFEAT, NHID], bf16, kind="ExternalInput").ap()
    W2 = nc.dram_tensor("W2", [NHID, NHID], bf16, kind="ExternalInput").ap()
    W3p = nc.dram_tensor("W3p", [NHID, NHID], bf16, kind="ExternalInput").ap()
    b1r = nc.dram_tensor("b1r", [P, NHID], f32, kind="ExternalInput").ap()
    b2r = nc.dram_tensor("b2r", [P, NHID], f32, kind="ExternalInput").ap()
    b3r = nc.dram_tensor("b3r", [P, NHID], f32, kind="ExternalInput").ap()
    ident = nc.dram_tensor("ident", [P, P], f32, kind="ExternalInput").ap()
    out = nc.dram_tensor("out", [NLOC, NCLASS], f32, kind="ExternalOutput").ap()

    rg = [list(range(NCORES))]

    with tile.TileContext(nc) as tc:
        with (
            tc.tile_pool(name="consts", bufs=1) as cp,
            tc.tile_pool(name="dram", bufs=1, space="DRAM") as dp,
            tc.tile_pool(name="msg", bufs=4) as mp,
            tc.tile_pool(name="hall", bufs=2) as hp,
            tc.tile_pool(name="supp", bufs=4) as supp,
            tc.tile_pool(name="soft", bufs=2) as sfp,
            tc.tile_pool(name="ps_dense", bufs=2, space="PSUM") as ps_dense,
            tc.tile_pool(name="ps_tr", bufs=2, space="PSUM") as ps_tr,
        ):
            # ---- constants ----
            idx_sb = cp.tile([P, 8 * SC], i16)
            ewp_sb = cp.tile([P, 4 * SC], bf16)
            ident_sb = cp.tile([P, P], f32)
            b1_sb = cp.tile([P, NHID], f32)
            b2_sb = cp.tile([P, NHID], f32)
            b3_sb = cp.tile([P, NHID], f32)
            W2_sb = cp.tile([NHID, NHID], bf16)
            W3_sb = cp.tile([NHID, NHID], bf16)
            nc.sync.dma_start(out=idx_sb[:], in_=idxt[:])
            nc.sync.dma_start(out=ewp_sb[:], in_=ewp[:])
            nc.sync.dma_start(out=ident_sb[:], in_=ident[:])
            nc.sync.dma_start(out=b1_sb[:], in_=b1r[:])
            nc.sync.dma_start(out=b2_sb[:], in_=b2r[:])
            nc.sync.dma_start(out=b3_sb[:], in_=b3r[:])
            nc.sync.dma_start(out=W2_sb[:], in_=W2[:])
            nc.sync.dma_start(out=W3_sb[:], in_=W3p[:])
            W1_sb = []
            for kc in range(4):
                w = cp.tile([P, NHID], bf16, name=f"W1_sb_{kc}")
                nc.sync.dma_start(out=w[:], in_=W1[kc * P:(kc + 1) * P, :])
                W1_sb.append(w)

            hT1_sb = cp.tile([NHID, NLOC], bf16)
            hT2_sb = cp.tile([NHID, NLOC], bf16)

            # ---- internal DRAM (tables viewed as pair rows) ----
            sup1_l = dp.tile([NLOC, NHID], bf16)
            sup2_l = dp.tile([NLOC, NHID], bf16)
            sup3_l = dp.tile([NLOC, NHID], bf16)
            T1 = dp.tile([NPAIR, 2 * NHID], bf16, addr_space="Shared")
            T2 = dp.tile([NPAIR, 2 * NHID], bf16, addr_space="Shared")
            T3 = dp.tile([NPAIR, 2 * NHID], bf16, addr_space="Shared")

            # ---- phase A: support1 = x @ W1 + b1 ----
            with tc.tile_pool(name="xchunk", bufs=8) as xp:
                for m in range(BLOCKS):
                    chunks = []
                    for kc in range(4):
                        s = xp.tile([P, P], bf16, name="xc")
                        nc.sync.dma_start(
                            out=s[:],
                            in_=xT[kc * P:(kc + 1) * P, m * P:(m + 1) * P])
                        chunks.append(s)
                    psd = ps_dense.tile([P, NHID], f32, space="PSUM")
                    for kc in range(4):
                        nc.tensor.matmul(
                            psd[:], lhsT=chunks[kc][:],
                            rhs=W1_sb[kc][:], start=(kc == 0), stop=(kc == 3))
                    sup = supp.tile([P, NHID], bf16)
                    nc.vector.tensor_tensor(out=sup[:], in0=psd[:], in1=b1_sb[:],
                                            op=OP.add)
                    nc.sync.dma_start(out=sup1_l[m * P:(m + 1) * P, :], in_=sup[:])

            nc.gpsimd.collective_compute(
                "AllGather", OP.bypass, replica_groups=rg,
                ins=[sup1_l.opt()], outs=[T1.opt()])

            JMAX = max(R * u for (R, u) in profile)
            RMAX = max(R for (R, u) in profile)

            def spmm(table, dout, post_group):
                """Pair-gather + parity-masked scale + tree segment-sum."""
                cofs = 0
                rstart = 0
                if not hasattr(spmm, "qn"):
                    spmm.qn = 0
                for (R, u) in profile:
                    J = R * u
                    msg = mp.tile([P, JMAX * 2 * NHID], bf16, name="msg_t")
                    nc.gpsimd.dma_gather(
                        out_ap=msg[:, :J * 2 * NHID]
                        .rearrange("p (j e) -> p j e", e=2 * NHID),
                        in_ap=table[:, :],
                        idxs_ap=idx_sb[:, 8 * cofs:8 * (cofs + J)],
                        num_idxs=P * J, num_idxs_reg=P * J,
                        elem_size=2 * NHID, single_packet=False,
                        queue_num=spmm.qn % 4)
                    spmm.qn += 1
                    # scale both halves by parity-masked duplicated weights
                    # (one 3-free-dim instruction per pair-half)
                    mj0 = msg[:, :J * 2 * NHID].rearrange(
                        "p (j e) -> p j e", e=2 * NHID)
                    for hh in range(2):
                        m4h = mj0[:, :, hh * NHID:hh * NHID + dout].rearrange(
                            "p j (f2 r) -> p j f2 r", r=2)
                        e4h = ewp_sb[:, 4 * cofs:4 * (cofs + J)].rearrange(
                            "p (j h r) -> p j h r", h=2, r=2)[
                            :, :, hh, :].unsqueeze(2).to_broadcast(
                            [P, J, dout // 2, 2])
                        nc.vector.tensor_tensor(out=m4h, in0=m4h, in1=e4h,
                                                op=OP.mult)
                    # combine halves: wanted half has the weight, other is 0
                    mj = msg[:, :J * 2 * NHID].rearrange(
                        "p (j e) -> p j e", e=2 * NHID)
                    nc.vector.tensor_tensor(
                        out=mj[:, :, 0:dout], in0=mj[:, :, 0:dout],
                        in1=mj[:, :, NHID:NHID + dout], op=OP.add)
                    # in-place pairwise tree over the u occurrence columns
                    m3 = msg[:, :J * 2 * NHID].rearrange(
                        "p (r j e) -> p r j e", r=R, j=u)
                    h = u
                    while h > 2:
                        k = h - h // 2
                        nc.vector.tensor_tensor(
                            out=m3[:, :, 0:h // 2, 0:dout],
                            in0=m3[:, :, 0:h // 2, 0:dout],
                            in1=m3[:, :, k:k + h // 2, 0:dout], op=OP.add)
                        h = k
                    h_all = hp.tile([P, RMAX * NHID], f32, name="h_all")
                    ha = h_all[:, :R * NHID].rearrange(
                        "p (r f) -> p r f", f=NHID)[:, :, 0:dout]
                    if h == 2:
                        nc.vector.tensor_tensor(
                            out=ha, in0=m3[:, :, 0, 0:dout],
                            in1=m3[:, :, 1, 0:dout], op=OP.add)
                    else:
                        nc.vector.tensor_scalar_mul(ha, m3[:, :, 0, 0:dout], 1.0)
                    post_group(rstart, R, h_all)
                    cofs += J
                    rstart += R

            def mk_post_h(hT_dst, func, Wn_sb, bn_sb, supn_l, dn):
                def post(rstart, R, h_all):
                    for i in range(R):
                        r = rstart + i
                        pst = ps_tr.tile([NHID, P], f32, space="PSUM",
                                         name="ps_t")
                        nc.tensor.transpose(
                            pst[:], h_all[:, i * NHID:(i + 1) * NHID],
                            ident_sb[:])
                        nc.scalar.activation(
                            hT_dst[:, r * P:(r + 1) * P], pst[:], func)
                        psd = ps_dense.tile([P, NHID], f32, space="PSUM")
                        nc.tensor.matmul(
                            psd[:], lhsT=hT_dst[:, r * P:(r + 1) * P],
                            rhs=Wn_sb[:], start=True, stop=True)
                        sup = supp.tile([P, NHID], bf16, name="sup_n")
                        nc.vector.tensor_tensor(
                            out=sup[:], in0=psd[:], in1=bn_sb[:], op=OP.add)
                        nc.sync.dma_start(
                            out=supn_l[r * P:(r + 1) * P, :], in_=sup[:])
                return post

            # ---- L1 SPMM + relu -> hT1, dense2 -> sup2 ----
            spmm(T1, NHID, mk_post_h(hT1_sb, AF.Relu, W2_sb, b2_sb,
                                     sup2_l, NHID))
            nc.gpsimd.collective_compute(
                "AllGather", OP.bypass, replica_groups=rg,
                ins=[sup2_l.opt()], outs=[T2.opt()])

            # ---- L2 SPMM -> hT2, dense3 (padded) -> sup3 ----
            spmm(T2, NHID, mk_post_h(hT2_sb, AF.Copy, W3_sb, b3_sb,
                                     sup3_l, NHID))
            nc.gpsimd.collective_compute(
                "AllGather", OP.bypass, replica_groups=rg,
                ins=[sup3_l.opt()], outs=[T3.opt()])

            # ---- L3 SPMM + log_softmax -> out ----
            def post3(rstart, R, h_all):
                h3 = h_all[:, :R * NHID].rearrange(
                    "p (r f) -> p r f", f=NHID)[:, :, 0:NCLASS]
                negmax = sfp.tile([P, RMAX], f32, name="negmax")
                nc.vector.reduce_max(out=negmax[:, :R], in_=h3, axis=AX,
                                     negate=True)
                t = sfp.tile([P, RMAX * NCLASS], f32, name="t_sm")
                tv = t[:, :R * NCLASS].rearrange("p (r f) -> p r f", f=NCLASS)
                nc.vector.tensor_tensor(
                    out=tv, in0=h3,
                    in1=negmax[:, :R].unsqueeze(2).to_broadcast([P, R, NCLASS]),
                    op=OP.add)
                e = sfp.tile([P, RMAX * NCLASS], f32, name="e_sm")
                nc.scalar.activation(e[:, :R * NCLASS], t[:, :R * NCLASS],
                                     AF.Exp)
                ssum = sfp.tile([P, RMAX], f32, name="ssum")
                nc.vector.reduce_sum(
                    out=ssum[:, :R],
                    in_=e[:, :R * NCLASS].rearrange("p (r f) -> p r f",
                                                    f=NCLASS),
                    axis=AX)
                lns = sfp.tile([P, RMAX], f32, name="lns")
                nc.scalar.activation(lns[:, :R], ssum[:, :R], AF.Ln)
                o = sfp.tile([P, RMAX * NCLASS], f32, name="o_sm")
                ov = o[:, :R * NCLASS].rearrange("p (r f) -> p r f", f=NCLASS)
                nc.vector.tensor_tensor(
                    out=ov, in0=tv,
                    in1=lns[:, :R].unsqueeze(2).to_broadcast([P, R, NCLASS]),
                    op=OP.subtract)
                nc.sync.dma_start(
                    out=out[rstart * P:(rstart + R) * P, :].rearrange(
                        "(r p) f -> p r f", p=P),
                    in_=ov)
            spmm(T3, NCLASS, post3)

    nc.compile()
    return nc


def kernel(x, edge_weight, W1, b1, W2, b2, W3, b3, row, col):
    from concourse import bass_utils

    x = np.asarray(x, np.float32)
    edge_weight = np.asarray(edge_weight, np.float32)
    row = np.asarray(row, np.int64)
    col = np.asarray(col, np.int64)

    perm, idx16, ewp4, profile, SC = _partition_graph(
        row, col, edge_weight)

    key = (profile, SC)
    if key not in _cache:
        _cache[key] = _build_program(profile, SC)
    nc = _cache[key]

    x_new = np.zeros((NTOT, NFEAT), np.float32)
    x_new[perm] = x
    ident = np.eye(P, dtype=np.float32)
    bf = ml_dtypes.bfloat16
    W3p = np.zeros((NHID, NHID), np.float32)
    W3p[:, :NCLASS] = np.asarray(W3, np.float32)
    b3p = np.zeros((NHID,), np.float32)
    b3p[:NCLASS] = np.asarray(b3, np.float32)
    in_maps = []
    for c in range(NCORES):
        in_maps.append({
            "xT": np.ascontiguousarray(
                x_new[c * NLOC:(c + 1) * NLOC].T).astype(bf),
            "idxt": np.ascontiguousarray(idx16[c]),
            "ewp": np.ascontiguousarray(ewp4[c]),
            "W1": np.asarray(W1, np.float32).astype(bf),
            "W2": np.asarray(W2, np.float32).astype(bf),
            "W3p": W3p.astype(bf),
            "b1r": np.tile(np.asarray(b1, np.float32), (P, 1)),
            "b2r": np.tile(np.asarray(b2, np.float32), (P, 1)),
            "b3r": np.tile(b3p, (P, 1)),
            "ident": ident,
        })

    res = bass_utils.run_bass_kernel_spmd(
        nc, in_maps, core_ids=list(range(NCORES)),
        trace=kernel.trace)
    kernel.last_result = res

    full = np.concatenate([res.results[c]["out"] for c in range(NCORES)],
                          axis=0)
    return full[perm].astype(np.float32)


kernel.trace = False
kernel.last_result = None
